# revision 10
# baseline (speedup 1.0000x reference)
"""AdaptGNN 3-layer message passing on 8 TRN2 NeuronCores.

Data-parallel over batch B=8: core c owns batch element c.

Math per core (N=2048, D=H=128):
  h = x
  for l in 0..2:
      hW  = h @ Wl + bl
      cos = normalize(hW) @ normalize(hW)^T
      h   = (ew * cos) @ hW      (+ relu except last layer)

Device-side formulation (all transposes / broadcasts hoisted to the host):
  - Host supplies xT = x^T (bf16), ewT = ew^T (bf16), W (bf16), and
    b pre-broadcast to [128,128] (f32).  Output is returned transposed
    plus the final layer's inv-norm vector; the host applies the last
    per-node scale and transposes back.
  - Loop state is hT_raw [128, N] (bf16, feature dim on partitions) with a
    deferred per-node scale s[n] from the previous layer's normalization
    (h_true[:, n] = s[n] * hT_raw[:, n]); s=1 for layer 0.
  - hW tile t: matmul(lhsT=hT_raw[:, t], W) -> psum; DVE
    scalar_tensor_tensor evac: hwn = (psum * s_col) + b_bc  (true hW, bf16).
  - Row norms: GPSIMD scalar_tensor_tensor hwn*hwn with accum_out -> n2;
    inv = 1/max(sqrt(n2), eps).
  - hWT via 16 PE tile transposes of hwn (bf16 psum, ACT evac).
  - Gram G[q,p] = hW[q]·hW[p]: matmul(lhsT=hWT qb-block, rhs=hWT j-chunk).
  - mt[q,p] = (G * inv[q]) * ewT[q,p], split across three producer paths to
    balance engines: DVE fused from psum / ACT evac + DVE mult / ACT evac +
    GPSIMD mult.
  - aggT_raw[c,p] += hwn[qb]^T @ mt  accumulated over qb in psum
    (= out[p,c] missing inv[p]; that factor is the next layer's s).
  - Grams for qb run one step ahead of aggs for qb-1 so the PE never waits
    on the mt producers.
"""

import functools

import numpy as np

N = 2048
D = 128
T = N // 128          # 16 row blocks
NCHUNK = N // 512     # 4 free-dim chunks for N=512 matmuls
N_CORES = 8
EPS = 1e-12

# mt producer split, cycle of 32 tiles: 'F' = DVE fused from psum (no SBUF
# 2-port use, immune to GPSIMD port contention), 'G' = ACT evac + GPSIMD
# mult.  13 G per 32 ~ balances DVE vs ACT vs GPSIMD.
_GSET = {0, 2, 5, 7, 10, 12, 15, 17, 20, 22, 25, 27, 30}
MT_PATTERN = "".join("G" if i in _GSET else "F" for i in range(32))


@functools.lru_cache(maxsize=1)
def build_nc():
    import concourse.bass as bass
    from concourse import bacc, masks, mybir, tile

    f32 = mybir.dt.float32
    bf16 = mybir.dt.bfloat16
    AF = mybir.ActivationFunctionType
    ALU = mybir.AluOpType

    nc = bacc.Bacc(None, target_bir_lowering=False)

    xT_d = nc.declare_dram_parameter("xT", [D, N], bf16, isOutput=False)
    ewT_d = nc.declare_dram_parameter("ewT", [N, N], bf16, isOutput=False)
    w_d = []
    b_d = []
    for l in range(3):
        w_d.append(nc.declare_dram_parameter(f"W{l}", [D, D], bf16, isOutput=False))
        b_d.append(nc.declare_dram_parameter(f"B{l}", [D, D], f32, isOutput=False))
    out_d = nc.declare_dram_parameter("out", [D, N], f32, isOutput=True)
    inv_d = nc.declare_dram_parameter("inv3", [128, T], f32, isOutput=True)

    with tile.TileContext(nc) as tc:
        with (
            tc.tile_pool(name="persist", bufs=1) as persist,
            tc.tile_pool(name="consts", bufs=1) as consts,
            tc.tile_pool(name="hts", bufs=2) as hts,
            tc.tile_pool(name="hwn_p", bufs=2) as hwn_p,
            tc.tile_pool(name="hwt_p", bufs=2) as hwt_p,
            tc.tile_pool(name="inv_p", bufs=2) as inv_p,
            tc.tile_pool(name="scr_p", bufs=4) as scr_p,
            tc.tile_pool(name="gs_p", bufs=12) as gs_p,
            tc.tile_pool(name="mt_p", bufs=16) as mt_p,
            tc.tile_pool(name="psum", bufs=4, space="PSUM") as psum,
        ):
            # ---- loads (HWDGE; no casts or transposes needed) ----
            hT0 = hts.tile([128, N], bf16, tag="hT", name="hT0")
            nc.sync.dma_start(hT0[:], xT_d[:, :])
            W_bf = []
            b_bc = []
            for l in range(3):
                wb = consts.tile([128, 128], bf16, tag=f"wbf{l}", name=f"wb{l}")
                nc.sync.dma_start(wb[:], w_d[l][:, :])
                W_bf.append(wb)
                bb = consts.tile([128, 128], f32, tag=f"bbc{l}", name=f"bb{l}")
                nc.sync.dma_start(bb[:], b_d[l][:, :])
                b_bc.append(bb)

            ident_bf = consts.tile([128, 128], bf16, tag="idb")
            masks.make_identity(nc, ident_bf[:])

            # ewT half-slabs, in (pass, qb) consumption order of the layer-0
            # two-pass main loop so compute never waits on the tail of the
            # 8 MiB load.
            ewT = persist.tile([128, T, N], bf16, tag="ewT")
            for half in range(2):
                for qb in range(T):
                    nc.sync.dma_start(
                        ewT[:, qb, half * 1024:(half + 1) * 1024],
                        ewT_d[qb * 128:(qb + 1) * 128,
                              half * 1024:(half + 1) * 1024],
                    )

            # ---- layers ----
            hT = hT0
            s_prev = None  # deferred per-node scale (None => 1.0)
            for l in range(3):
                hwn = hwn_p.tile([128, T, 128], bf16, tag="hwn")
                hWT = hwt_p.tile([128, N], bf16, tag="hWT")
                n2 = inv_p.tile([128, T], f32, tag="n2", name=f"n2_{l}")
                inv = inv_p.tile([128, T], f32, tag="inv", name=f"inv_{l}")

                # hW + transposes, software-pipelined so the PE isn't gated
                # on the step-2 evac latency tile by tile.
                def emit_hw(t):
                    ps = psum.tile([128, 512], f32, tag="g", bufs=6)
                    nc.tensor.matmul(
                        ps[:, 0:128], hT[:, t * 128:(t + 1) * 128], W_bf[l][:]
                    )
                    if s_prev is None:
                        nc.vector.tensor_add(hwn[:, t, :], ps[:, 0:128], b_bc[l][:])
                    else:
                        nc.vector.scalar_tensor_tensor(
                            hwn[:, t, :], ps[:, 0:128], s_prev[:, t:t + 1],
                            b_bc[l][:], op0=ALU.mult, op1=ALU.add,
                        )
                    sq = scr_p.tile([128, 128], bf16, tag="sq", bufs=4)
                    nc.scalar.activation(
                        sq[:], hwn[:, t, :], AF.Square,
                        accum_out=n2[:, t:t + 1],
                    )

                def emit_tr(t):
                    ps2 = psum.tile([128, 512], bf16, tag="g", bufs=6)
                    nc.tensor.transpose(ps2[:, 0:128], hwn[:, t, :], ident_bf[:])
                    dst = hWT[:, t * 128:(t + 1) * 128]
                    if t % 2 == 0:
                        nc.scalar.activation(dst, ps2[:, 0:128], AF.Copy)
                    else:
                        nc.vector.tensor_copy(dst, ps2[:, 0:128])

                for t in range(4):
                    emit_hw(t)
                for t in range(4, T):
                    emit_hw(t)
                    emit_tr(t - 4)
                for t in range(T - 4, T):
                    emit_tr(t)

                # inv = 1 / max(sqrt(n2), eps)
                nrm = inv_p.tile([128, T], f32, tag="nrm", name=f"nrm_{l}")
                nc.scalar.activation(nrm[:], n2[:], AF.Sqrt)
                nc.vector.tensor_scalar_max(nrm[:], nrm[:], EPS)
                nc.vector.reciprocal(inv[:], nrm[:])

                # main loop: two passes over chunk pairs so only 2 agg banks
                # are live and the gram pool gets 6 PSUM banks; grams for qb
                # run one step ahead of aggs for qb-1 so the PE never waits
                # on the mt producers.
                if l == 2:
                    nc.sync.dma_start(inv_d[:, :], inv[:])
                    aggs = persist.tile([128, N], f32, tag="aggs")
                if l < 2:
                    hT_next = hts.tile(
                        [128, N], bf16, tag="hT", name=f"hT{l + 1}"
                    )
                mt_idx = 0
                for p in range(2):
                    chunks = (2 * p, 2 * p + 1)
                    agg = {
                        j: psum.tile(
                            [128, 512], f32, tag="agg", bufs=2,
                            name=f"agg{l}_{j}",
                        )
                        for j in chunks
                    }

                    def emit_gram(qb):
                        tiles = {}
                        for j in chunks:
                            g_ps = psum.tile([128, 512], f32, tag="g", bufs=6)
                            nc.tensor.matmul(
                                g_ps[:],
                                hWT[:, qb * 128:(qb + 1) * 128],
                                hWT[:, j * 512:(j + 1) * 512],
                            )
                            tiles[j] = g_ps
                        return tiles

                    def emit_mt_agg(qb, g_tiles):
                        nonlocal mt_idx
                        for j in chunks:
                            g_ps = g_tiles[j]
                            ew_sl = ewT[:, qb, j * 512:(j + 1) * 512]
                            mt = mt_p.tile([128, 512], bf16, tag="mt")
                            path = MT_PATTERN[mt_idx % len(MT_PATTERN)]
                            mt_idx += 1
                            if path == "F":
                                nc.vector.scalar_tensor_tensor(
                                    mt[:], g_ps[:], inv[:, qb:qb + 1], ew_sl,
                                    op0=ALU.mult, op1=ALU.mult,
                                )
                            else:
                                gs = gs_p.tile([128, 512], bf16, tag="gs")
                                nc.scalar.activation(
                                    gs[:], g_ps[:], AF.Copy,
                                    scale=inv[:, qb:qb + 1],
                                )
                                nc.gpsimd.tensor_tensor(
                                    mt[:], gs[:], ew_sl, op=ALU.mult
                                )
                            nc.tensor.matmul(
                                agg[j][:], hwn[:, qb, :], mt[:],
                                start=(qb == 0), stop=(qb == T - 1),
                            )

                    prev = emit_gram(0)
                    for qb in range(1, T):
                        cur = emit_gram(qb)
                        emit_mt_agg(qb - 1, prev)
                        prev = cur
                    emit_mt_agg(T - 1, prev)

                    for j in chunks:
                        if l < 2:
                            nc.scalar.activation(
                                hT_next[:, j * 512:(j + 1) * 512],
                                agg[j][:], AF.Relu,
                            )
                        else:
                            nc.scalar.activation(
                                aggs[:, j * 512:(j + 1) * 512],
                                agg[j][:], AF.Copy,
                            )
                    if l == 2:
                        nc.sync.dma_start(
                            out_d[:, p * 1024:(p + 1) * 1024],
                            aggs[:, p * 1024:(p + 1) * 1024],
                        )

                if l < 2:
                    hT = hT_next
                    s_prev = inv

    nc.compile()
    return nc


def prepare_in_maps(x, ew, params):
    """Host-side input transform: transposes, casts, bias broadcast."""
    import ml_dtypes

    bf16 = ml_dtypes.bfloat16
    common = {}
    for l in range(3):
        common[f"W{l}"] = np.ascontiguousarray(
            params[f"W{l}"].astype(bf16)
        )
        common[f"B{l}"] = np.ascontiguousarray(
            np.broadcast_to(
                params[f"b{l}"].astype(np.float32)[None, :], (128, 128)
            )
        )
    in_maps = []
    for c in range(N_CORES):
        in_maps.append({
            "xT": np.ascontiguousarray(x[c].T.astype(bf16)),
            "ewT": np.ascontiguousarray(ew[c].T.astype(bf16)),
            **common,
        })
    return in_maps


def assemble_output(results):
    """Host-side output transform: apply deferred inv scale, transpose."""
    out = np.empty((N_CORES, N, D), dtype=np.float32)
    for c in range(N_CORES):
        aggT = results[c]["out"]          # [D, N], missing inv3[n] scale
        inv3 = results[c]["inv3"]         # [128, T]; n = t*128 + p
        s = inv3.T.reshape(N)             # s[n]
        out[c] = aggT.T * s[:, None]
    return out


def kernel(**inputs):
    from concourse.bass_utils import run_bass_kernel_spmd

    x = np.asarray(inputs["x"], dtype=np.float32)
    ew = np.asarray(inputs["edge_weight"], dtype=np.float32)
    params = {}
    for l in range(3):
        params[f"W{l}"] = np.asarray(inputs[f"W{l}"], dtype=np.float32)
        params[f"b{l}"] = np.asarray(inputs[f"b{l}"], dtype=np.float32)

    nc = build_nc()
    in_maps = prepare_in_maps(x, ew, params)
    res = run_bass_kernel_spmd(nc, in_maps, core_ids=list(range(N_CORES)))
    return assemble_output(res.results)


# revision 11
# speedup vs baseline: 1.1427x; 1.1427x over previous
"""AdaptGNN 3-layer message passing on 8 TRN2 NeuronCores.

Data-parallel over batch B=8: core c owns batch element c.

Math per core (N=2048, D=H=128):
  h = x
  for l in 0..2:
      hW  = h @ Wl + bl
      cos = normalize(hW) @ normalize(hW)^T
      h   = (ew * cos) @ hW      (+ relu except last layer)

Device-side formulation (all transposes / broadcasts hoisted to the host):
  - Host supplies xT = x^T (bf16), ewT = ew^T (bf16), W (bf16); output is
    returned transposed plus the final layer's inv-norm vector; the host
    applies the last per-node scale and transposes back.
  - Loop state is hT_raw [128, N] (bf16, feature dim on partitions) with a
    deferred per-node scale s[n] from the previous layer's normalization
    (h_true[:, n] = s[n] * hT_raw[:, n]); s=1 for layer 0.
  - hW tile t: matmul(lhsT=hT_raw[:, t], W) -> psum; evac applies s (and
    bias when nonzero): hwn = psum * s_col (+ b_bc), split DVE/ACT when
    bias-free.  ACT Square(+accum) on hwn -> row norms -> inv.
  - hWT via 16 PE tile transposes of hwn (bf16 psum, ACT/DVE evac).
  - Gram G[q,p] = hW[q]·hW[p]: matmul(lhsT=hWT qb-block, rhs=hWT j-chunk).
  - mt[q,p] = (G * inv[q]) * ewT[q,p], split across producer paths to
    balance engines: DVE fused from psum ('F'), ACT evac + DVE mult ('S'),
    ACT evac + GPSIMD mult ('G').
  - aggT_raw[c,p] += hwn[qb]^T @ mt accumulated over qb in psum
    (= out[p,c] missing inv[p]; that factor is the next layer's s).
  - Main loop is chunk-major (4 passes of 16 qb per layer) so each output
    chunk finishes early: its relu evac and the NEXT layer's hW/norm/
    transpose prologue for the matching 4 tiles run overlapped with the
    remaining passes, keeping the PE busy across layer boundaries.
  - Grams for qb run one step ahead of aggs for qb-1 so the PE never waits
    on the mt producers.
"""

import functools

import numpy as np

N = 2048
D = 128
T = N // 128          # 16 row blocks
NCHUNK = N // 512     # 4 free-dim chunks for N=512 matmuls
N_CORES = 8
EPS = 1e-12

# mt producer split, cycle of 32 tiles: 'F' = DVE fused from psum (no SBUF
# 2-port use), 'S' = ACT evac + DVE mult, 'G' = ACT evac + GPSIMD mult.
# 19F/3S/10G per 32 balances DVE vs ACT vs GPSIMD without saturating the
# shared GPSIMD/DVE SBUF ports.
_GSET = {0, 3, 6, 9, 12, 16, 19, 22, 25, 28}
_SSET = {7, 14, 30}
MT_PATTERN = "".join(
    "G" if i in _GSET else ("S" if i in _SSET else "F") for i in range(32)
)


@functools.lru_cache(maxsize=2)
def build_nc(has_bias=False):
    import concourse.bass as bass
    from concourse import bacc, masks, mybir, tile

    f32 = mybir.dt.float32
    bf16 = mybir.dt.bfloat16
    AF = mybir.ActivationFunctionType
    ALU = mybir.AluOpType

    nc = bacc.Bacc(None, target_bir_lowering=False)

    xT_d = nc.declare_dram_parameter("xT", [D, N], bf16, isOutput=False)
    ewT_d = nc.declare_dram_parameter("ewT", [N, N], bf16, isOutput=False)
    w_d = []
    b_d = []
    for l in range(3):
        w_d.append(nc.declare_dram_parameter(f"W{l}", [D, D], bf16, isOutput=False))
        b_d.append(nc.declare_dram_parameter(f"B{l}", [D, D], f32, isOutput=False))
    out_d = nc.declare_dram_parameter("out", [D, N], f32, isOutput=True)
    inv_d = nc.declare_dram_parameter("inv3", [128, T], f32, isOutput=True)

    with tile.TileContext(nc) as tc:
        with (
            tc.tile_pool(name="persist", bufs=1) as persist,
            tc.tile_pool(name="consts", bufs=1) as consts,
            tc.tile_pool(name="hts", bufs=2) as hts,
            tc.tile_pool(name="hwn_p", bufs=2) as hwn_p,
            tc.tile_pool(name="hwt_p", bufs=2) as hwt_p,
            tc.tile_pool(name="inv_p", bufs=2) as inv_p,
            tc.tile_pool(name="scr_p", bufs=4) as scr_p,
            tc.tile_pool(name="gs_p", bufs=12) as gs_p,
            tc.tile_pool(name="mt_p", bufs=16) as mt_p,
            tc.tile_pool(name="psum", bufs=6, space="PSUM") as psum,
        ):
            # ---- loads (HWDGE; no casts or transposes needed) ----
            hT0 = hts.tile([128, N], bf16, tag="hT", name="hT0")
            nc.sync.dma_start(hT0[:], xT_d[:, :])
            W_bf = []
            b_bc = []
            for l in range(3):
                wb = consts.tile([128, 128], bf16, tag=f"wbf{l}", name=f"wb{l}")
                nc.sync.dma_start(wb[:], w_d[l][:, :])
                W_bf.append(wb)
                if has_bias:
                    bb = consts.tile([128, 128], f32, tag=f"bbc{l}", name=f"bb{l}")
                    nc.sync.dma_start(bb[:], b_d[l][:, :])
                    b_bc.append(bb)
                else:
                    b_bc.append(None)

            ident_bf = consts.tile([128, 128], bf16, tag="idb")
            masks.make_identity(nc, ident_bf[:])

            # ewT quarter-slabs in (chunk, qb) consumption order of the
            # layer-0 chunk-major main loop, so compute never waits on the
            # tail of the 8 MiB load.
            ewT = persist.tile([128, T, N], bf16, tag="ewT")
            for j in range(NCHUNK):
                for qb in range(T):
                    nc.sync.dma_start(
                        ewT[:, qb, j * 512:(j + 1) * 512],
                        ewT_d[qb * 128:(qb + 1) * 128, j * 512:(j + 1) * 512],
                    )

            # ---- per-layer state ----
            hT = [hT0, None, None]
            hwn = [None] * 3
            hWT = [None] * 3
            n2 = [None] * 3
            inv = [None] * 3

            def alloc_layer(l):
                hwn[l] = hwn_p.tile([128, T, 128], bf16, tag="hwn",
                                    name=f"hwn{l}")
                hWT[l] = hwt_p.tile([128, N], bf16, tag="hWT", name=f"hWT{l}")
                n2[l] = inv_p.tile([128, T], f32, tag="n2", name=f"n2_{l}")
                inv[l] = inv_p.tile([128, T], f32, tag="inv", name=f"inv_{l}")

            def emit_hw(l, t):
                """hW matmul + evac (s/bias) + Square for tile t of layer l."""
                ps = psum.tile([128, 512], f32, tag="g", bufs=6)
                nc.tensor.matmul(
                    ps[:, 0:128], hT[l][:, t * 128:(t + 1) * 128], W_bf[l][:]
                )
                dst = hwn[l][:, t, :]
                s_col = inv[l - 1][:, t:t + 1] if l > 0 else None
                if has_bias:
                    if s_col is None:
                        nc.vector.tensor_add(dst, ps[:, 0:128], b_bc[l][:])
                    else:
                        nc.vector.scalar_tensor_tensor(
                            dst, ps[:, 0:128], s_col, b_bc[l][:],
                            op0=ALU.mult, op1=ALU.add,
                        )
                else:
                    # bias-free: split the evac between DVE and ACT
                    if t % 2 == 0:
                        if s_col is None:
                            nc.vector.tensor_copy(dst, ps[:, 0:128])
                        else:
                            nc.vector.tensor_scalar_mul(dst, ps[:, 0:128], s_col)
                    else:
                        if s_col is None:
                            nc.scalar.activation(dst, ps[:, 0:128], AF.Copy)
                        else:
                            nc.scalar.activation(
                                dst, ps[:, 0:128], AF.Copy, scale=s_col
                            )
                sq = scr_p.tile([128, 128], bf16, tag="sq", bufs=4)
                nc.scalar.activation(
                    sq[:], dst, AF.Square, accum_out=n2[l][:, t:t + 1]
                )

            def emit_tr(l, t):
                ps2 = psum.tile([128, 512], bf16, tag="g", bufs=6)
                nc.tensor.transpose(ps2[:, 0:128], hwn[l][:, t, :], ident_bf[:])
                dst = hWT[l][:, t * 128:(t + 1) * 128]
                if t % 2 == 0:
                    nc.scalar.activation(dst, ps2[:, 0:128], AF.Copy)
                else:
                    nc.vector.tensor_copy(dst, ps2[:, 0:128])

            def emit_norm_finalize(l):
                nrm = inv_p.tile([128, T], f32, tag="nrm", name=f"nrm_{l}")
                nc.scalar.activation(nrm[:], n2[l][:], AF.Sqrt)
                nc.vector.tensor_scalar_max(nrm[:], nrm[:], EPS)
                nc.vector.reciprocal(inv[l][:], nrm[:])

            # ---- layer-0 prologue (standalone; later prologues overlap
            # the previous layer's main loop) ----
            alloc_layer(0)
            for t in range(4):
                emit_hw(0, t)
            for t in range(4, T):
                emit_hw(0, t)
                emit_tr(0, t - 4)
            for t in range(T - 4, T):
                emit_tr(0, t)

            # ---- layers: chunk-major main loops ----
            mt_idx = 0
            aggs = persist.tile([128, N], f32, tag="aggs")
            for l in range(3):
                emit_norm_finalize(l)
                if l == 2:
                    nc.sync.dma_start(inv_d[:, :], inv[2][:])
                if l < 2:
                    hT[l + 1] = hts.tile(
                        [128, N], bf16, tag="hT", name=f"hT{l + 1}"
                    )
                    alloc_layer(l + 1)

                for j in range(NCHUNK):
                    agg = psum.tile(
                        [128, 512], f32, tag="agg", bufs=2, name=f"agg{l}_{j}"
                    )

                    def emit_gram(qb):
                        g_ps = psum.tile([128, 512], f32, tag="g", bufs=6)
                        nc.tensor.matmul(
                            g_ps[:],
                            hWT[l][:, qb * 128:(qb + 1) * 128],
                            hWT[l][:, j * 512:(j + 1) * 512],
                        )
                        return g_ps

                    def emit_mt_agg(qb, g_ps):
                        nonlocal mt_idx
                        ew_sl = ewT[:, qb, j * 512:(j + 1) * 512]
                        mt = mt_p.tile([128, 512], bf16, tag="mt")
                        path = MT_PATTERN[mt_idx % len(MT_PATTERN)]
                        mt_idx += 1
                        if path == "F":
                            nc.vector.scalar_tensor_tensor(
                                mt[:], g_ps[:], inv[l][:, qb:qb + 1], ew_sl,
                                op0=ALU.mult, op1=ALU.mult,
                            )
                        else:
                            gs = gs_p.tile([128, 512], bf16, tag="gs")
                            nc.scalar.activation(
                                gs[:], g_ps[:], AF.Copy,
                                scale=inv[l][:, qb:qb + 1],
                            )
                            eng = nc.vector if path == "S" else nc.gpsimd
                            eng.tensor_tensor(mt[:], gs[:], ew_sl, op=ALU.mult)
                        nc.tensor.matmul(
                            agg[:], hwn[l][:, qb, :], mt[:],
                            start=(qb == 0), stop=(qb == T - 1),
                        )

                    prev = emit_gram(0)
                    for qb in range(1, T):
                        cur = emit_gram(qb)
                        emit_mt_agg(qb - 1, prev)
                        prev = cur
                    emit_mt_agg(T - 1, prev)

                    # pass tail: evacuate this chunk, then overlap the next
                    # layer's prologue tiles for the matching row blocks.
                    if l < 2:
                        nc.scalar.activation(
                            hT[l + 1][:, j * 512:(j + 1) * 512],
                            agg[:], AF.Relu,
                        )
                        for t in range(4 * j, 4 * j + 4):
                            emit_hw(l + 1, t)
                        for t in range(4 * j, 4 * j + 4):
                            emit_tr(l + 1, t)
                    else:
                        nc.scalar.activation(
                            aggs[:, j * 512:(j + 1) * 512], agg[:], AF.Copy
                        )
                        nc.sync.dma_start(
                            out_d[:, j * 512:(j + 1) * 512],
                            aggs[:, j * 512:(j + 1) * 512],
                        )

    nc.compile()
    return nc


def prepare_in_maps(x, ew, params):
    """Host-side input transform: transposes, casts, bias broadcast."""
    import ml_dtypes

    bf16 = ml_dtypes.bfloat16
    common = {}
    for l in range(3):
        common[f"W{l}"] = np.ascontiguousarray(
            params[f"W{l}"].astype(bf16)
        )
        common[f"B{l}"] = np.ascontiguousarray(
            np.broadcast_to(
                params[f"b{l}"].astype(np.float32)[None, :], (128, 128)
            )
        )
    in_maps = []
    for c in range(N_CORES):
        in_maps.append({
            "xT": np.ascontiguousarray(x[c].T.astype(bf16)),
            "ewT": np.ascontiguousarray(ew[c].T.astype(bf16)),
            **common,
        })
    return in_maps


def has_bias(params):
    return any(np.any(params[f"b{l}"]) for l in range(3))


def assemble_output(results):
    """Host-side output transform: apply deferred inv scale, transpose."""
    out = np.empty((N_CORES, N, D), dtype=np.float32)
    for c in range(N_CORES):
        aggT = results[c]["out"]          # [D, N], missing inv3[n] scale
        inv3 = results[c]["inv3"]         # [128, T]; n = t*128 + p
        s = inv3.T.reshape(N)             # s[n]
        out[c] = aggT.T * s[:, None]
    return out


def kernel(**inputs):
    from concourse.bass_utils import run_bass_kernel_spmd

    x = np.asarray(inputs["x"], dtype=np.float32)
    ew = np.asarray(inputs["edge_weight"], dtype=np.float32)
    params = {}
    for l in range(3):
        params[f"W{l}"] = np.asarray(inputs[f"W{l}"], dtype=np.float32)
        params[f"b{l}"] = np.asarray(inputs[f"b{l}"], dtype=np.float32)

    nc = build_nc(has_bias=has_bias(params))
    in_maps = prepare_in_maps(x, ew, params)
    res = run_bass_kernel_spmd(nc, in_maps, core_ids=list(range(N_CORES)))
    return assemble_output(res.results)
